# revision 11
# baseline (speedup 1.0000x reference)
"""Causal sliding-window attention (T=8192, H=16, HK=4, D=128, W=512) on 8 trn2 cores.

Sharding: tensor-parallel on heads. Core c computes query heads {2c, 2c+1},
which share kv head c//2 (G = H//HK = 4, so 2 heads per core never straddle
a kv group). Each core is fully independent -- no collectives.

Per-core program (Bass/Tile, SPMD):
  inputs (host pre-transposed, pre-cast bf16):
    qT  [2*128, T]   Q^T per head (row block j = head j)
    kT  [128, T]     K^T of the shared kv head
    va  [128, nT*129] V chunks [128, 129] with a ones column (chunk t at cols
                     129t..129t+129); the ones column makes the PV matmul also
                     produce the softmax denominator.
  loop over k-chunks t (128 keys each), keys on PSUM partitions:
    S^T[rk, q] = kT_chunk.T @ qT  over the 640-wide valid q-span [128t, 128t+640)
    one ACT exp (scale=D^-0.5 folded in), fp32->bf16, into an SBUF ring
    triangular edge masks (diag block on DVE, border block on GpSimd)
    PV: for j=0..4, lhsT = E block (q-chunk t+j), rhs = va chunk t -> accumulate
        O_aug[q-chunk] = [128, 129] in PSUM (col 128 = denominator)
    retire q-chunk t: copy unnormalized O and denominator to SBUF staging.
  Normalization (out/den) and lse (log den) happen on the host.

The emission is software-pipelined (S matmuls run 2 k-steps ahead of exp) so
the in-order PE queue never stalls on ACT: this keeps PE busy continuously,
which also keeps the PE HAM clock-gate at 2.4 GHz.

PSUM (8 banks): 4 banks = S ring-of-3 [128, 1920]; 4 banks = O accumulators,
5 live slots with staggered lifetimes packed 2 per bank at offsets 0/129 using
(u, u+4) co-tenancy: bank u%4, offset (u//4)%2. start=True (whole-bank
has_written clear) is only ever issued by the offset-0 occupant at its first
touch, which is exactly when the other offset's previous occupant has retired.
"""

import numpy as np
import ml_dtypes

import concourse.bacc as bacc
import concourse.bass as bass
import concourse.mybir as mybir
import concourse.tile as tile
from concourse.bass_utils import run_bass_kernel_spmd

T, H, HK, D, W = 8192, 16, 4, 128, 512
NCORES = 8
SCALE = float(D) ** -0.5
BF16 = ml_dtypes.bfloat16
F32 = mybir.dt.float32
BF = mybir.dt.bfloat16

_NC_CACHE = {}


def _split_at_banks(col0, width):
    """Split [col0, col0+width) PSUM cols at 512 boundaries."""
    pieces = []
    c = col0
    end = col0 + width
    while c < end:
        nxt = min(end, (c // 512 + 1) * 512)
        pieces.append((c, nxt - c))
        c = nxt
    return pieces


def build_program(t_tokens=T):
    nT = t_tokens // 128  # number of 128-wide k/q chunks
    assert t_tokens % 512 == 0 and nT >= 8

    nc = bacc.Bacc("TRN2", target_bir_lowering=False, debug=False,
                   num_devices=NCORES)
    qT_d = nc.dram_tensor("qT", [2 * D, t_tokens], BF, kind="ExternalInput")
    kT_d = nc.dram_tensor("kT", [D, t_tokens], BF, kind="ExternalInput")
    va_d = nc.dram_tensor("va", [D, nT * 129], BF, kind="ExternalInput")
    oaug_d = nc.dram_tensor("oaug", [2, nT // 4, D, 516], F32,
                            kind="ExternalOutput")

    with tile.TileContext(nc) as tc:
        with (
            tc.tile_pool(name="resident", bufs=1) as rpool,
            tc.tile_pool(name="ostg", bufs=2) as ostg_pool,
            tc.tile_pool(name="psum", bufs=1, space="PSUM") as psum_pool,
        ):
            kT = rpool.tile([128, t_tokens], BF, tag="kT")
            va = rpool.tile([128, nT * 129], BF, tag="va")
            qT = [rpool.tile([128, t_tokens], BF, tag=f"qT{j}", name=f"qT{j}")
                  for j in range(2)]
            maskadd = rpool.tile([128, 256], BF, tag="maskadd")
            ident = rpool.tile([128, 128], BF, tag="ident")
            ering = rpool.tile([128, 3840], BF, tag="ering")

            sring = psum_pool.tile([128, 2048], F32, tag="sring")
            oacc = psum_pool.tile([128, 2048], F32, tag="oacc")

            # --- constants for PE-side masking ---
            # The triangular edge masks are applied as matmul accumulations
            # onto the S tiles: S_region += maskadd.T @ I adds -1e9 at the
            # invalid positions, so exp underflows them to exactly 0 and no
            # vector-engine masking is needed.
            # maskadd cols [0,128):  A[k, m] = -1e9 where m > k  (diag block)
            # maskadd cols [128,256): B[k, m] = -1e9 where m <= k (border)
            NEG = -1.0e9
            nc.gpsimd.memset(maskadd[:, :], 0.0)
            nc.gpsimd.affine_select(
                out=maskadd[:, 0:128], in_=maskadd[:, 0:128],
                compare_op=mybir.AluOpType.is_ge, fill=NEG,
                base=0, channel_multiplier=1, pattern=[[-1, 128]],
            )
            nc.gpsimd.affine_select(
                out=maskadd[:, 128:256], in_=maskadd[:, 128:256],
                compare_op=mybir.AluOpType.is_ge, fill=NEG,
                base=-1, channel_multiplier=-1, pattern=[[1, 128]],
            )
            nc.gpsimd.memset(ident[:, :], 1.0)
            nc.gpsimd.affine_select(
                out=ident[:, :], in_=ident[:, :],
                compare_op=mybir.AluOpType.is_equal, fill=0.0,
                base=0, channel_multiplier=1, pattern=[[-1, 128]],
            )

            # --- input DMA, sliced and interleaved in first-use order ---
            ks = t_tokens // 4
            vs = (nT * 129) // 4
            qs = t_tokens // 8
            for s in range(4):
                nc.sync.dma_start(kT[:, s * ks:(s + 1) * ks],
                                  kT_d[:, s * ks:(s + 1) * ks])
                nc.sync.dma_start(qT[0][:, s * qs:(s + 1) * qs],
                                  qT_d[0:128, s * qs:(s + 1) * qs])
                nc.sync.dma_start(va[:, s * vs:(s + 1) * vs],
                                  va_d[:, s * vs:(s + 1) * vs])
            for s in range(4, 8):
                nc.sync.dma_start(qT[0][:, s * qs:(s + 1) * qs],
                                  qT_d[0:128, s * qs:(s + 1) * qs])
            for s in range(8):
                nc.sync.dma_start(qT[1][:, s * qs:(s + 1) * qs],
                                  qT_d[128:256, s * qs:(s + 1) * qs])

            def s_matmuls(j, t):
                """S^T matmuls for k-chunk t into ring slice t%2.

                The 1024-col ring stride keeps the two slices in disjoint
                PSUM bank pairs, so the exp read of slice t%2 never shares a
                bank with the pipelined-ahead S write of slice (t+1)%2.
                Main and tail matmuls stay open (stop=False) so the
                triangular -1e9 mask-add matmuls can accumulate into them
                and close the groups.
                """
                base = 1024 * (t % 2)
                k0 = 128 * t
                wm = min(512, t_tokens - k0)
                nc.tensor.matmul(sring[:, base:base + wm],
                                 kT[:, k0:k0 + 128],
                                 qT[j][:, k0:k0 + wm],
                                 start=True, stop=False)
                nc.tensor.matmul(sring[:, base:base + 128],
                                 maskadd[:, 0:128], ident[:, :],
                                 start=False, stop=True)
                wt = min(128, max(0, t_tokens - k0 - 512))
                if wt > 0:
                    c0 = base + 512
                    nc.tensor.matmul(sring[:, c0:c0 + wt],
                                     kT[:, k0:k0 + 128],
                                     qT[j][:, k0 + 512:k0 + 512 + wt],
                                     start=True, stop=False)
                    nc.tensor.matmul(sring[:, c0:c0 + wt],
                                     maskadd[:, 128:256], ident[:, 0:wt],
                                     start=False, stop=True)
                return wm + wt

            def width_of(t):
                k0 = 128 * t
                return min(512, t_tokens - k0) + min(128, max(0, t_tokens - k0 - 512))

            def emit_exp(t):
                sbase = 1024 * (t % 2)
                ebase = 640 * (t % 6)
                w = width_of(t)
                nc.scalar.activation(ering[:, ebase:ebase + w],
                                     sring[:, sbase:sbase + w],
                                     mybir.ActivationFunctionType.Exp,
                                     scale=SCALE)

            def mask_and_pv(j, t):
                base = 640 * (t % 6)
                jmax = min(4, nT - 1 - t)
                # ascending j: same-bank co-tenancy requires the retiring
                # slot's last accumulation (j0) before the whole-bank
                # clearing start=True of slot u+4 (j4).
                for jj in range(jmax + 1):
                    u = t + jj
                    off = 512 * (u % 4) + 129 * ((u // 4) % 2)
                    first = (jj == 4) or (t == 0)
                    start = first and ((u // 4) % 2 == 0)
                    # Co-tenant accumulators share banks; the sim's
                    # bank-granular group checker can't express this, but its
                    # per-byte pending-zero value model (== HW has_written)
                    # verifies the numerics.
                    nc.tensor.matmul(
                        oacc[:, off:off + 129],
                        ering[:, base + 128 * jj:base + 128 * jj + 128],
                        va[:, 129 * t:129 * t + 129],
                        start=start, stop=(jj == 0),
                        skip_group_check=True)

            ostage = [None]

            def retire(j, t):
                u = t
                off = 512 * (u % 4) + 129 * ((u // 4) % 2)
                if t % 4 == 0:
                    ostage[0] = ostg_pool.tile([128, 516], F32, tag="ostage",
                                               name="ostage")
                nc.vector.tensor_copy(
                    ostage[0][:, 129 * (t % 4):129 * (t % 4) + 129],
                    oacc[:, off:off + 129])
                if t % 4 == 3:
                    nc.sync.dma_start(oaug_d[j, t // 4], ostage[0][:, :])

            # software-pipelined emission: S runs 1 k-step ahead of exp
            for j in range(2):
                s_matmuls(j, 0)
                for t in range(nT):
                    emit_exp(t)
                    if t + 1 < nT:
                        s_matmuls(j, t + 1)
                    mask_and_pv(j, t)
                    retire(j, t)

    nc.compile()
    return nc


def _get_nc(t_tokens=T):
    if t_tokens not in _NC_CACHE:
        _NC_CACHE[t_tokens] = build_program(t_tokens)
    return _NC_CACHE[t_tokens]


def make_in_maps(query, key, value, t_tokens=T):
    q = np.asarray(query).astype(BF16).reshape(t_tokens, H, D)
    k = np.asarray(key).astype(BF16).reshape(t_tokens, HK, D)
    v = np.asarray(value).astype(BF16).reshape(t_tokens, HK, D)
    nT = t_tokens // 128
    in_maps = []
    for c in range(NCORES):
        h0, hk = 2 * c, c // 2
        qT = np.ascontiguousarray(
            q[:, h0:h0 + 2, :].transpose(1, 2, 0)).reshape(2 * D, t_tokens)
        kT = np.ascontiguousarray(k[:, hk, :].T)
        vv = v[:, hk, :].reshape(nT, 128, D).transpose(1, 0, 2)
        va = np.empty((128, nT, D + 1), dtype=BF16)
        va[:, :, :D] = vv
        va[:, :, D] = 1.0
        in_maps.append({"qT": qT, "kT": kT, "va": va.reshape(128, nT * 129)})
    return in_maps


def assemble(results, t_tokens=T):
    nT = t_tokens // 128
    out = np.empty((t_tokens, H * D), np.float32)
    lse = np.empty((H, t_tokens), np.float32)
    for c in range(NCORES):
        oaug = results[c]["oaug"]  # [2, nT//4, 128, 516]
        for j in range(2):
            a = oaug[j].reshape(nT // 4, 128, 4, 129)
            a = a.transpose(0, 2, 1, 3).reshape(t_tokens, 129)
            d_q = a[:, 128]
            cols = slice(256 * c + 128 * j, 256 * c + 128 * j + 128)
            out[:, cols] = a[:, :128] / d_q[:, None]
            lse[2 * c + j] = np.log(d_q)
    return out, lse


def kernel(query, key, value):
    nc = _get_nc(T)
    in_maps = make_in_maps(query, key, value, T)
    res = run_bass_kernel_spmd(nc, in_maps, list(range(NCORES)))
    return assemble(res.results, T)


# revision 12
# speedup vs baseline: 1.5330x; 1.5330x over previous
"""Causal sliding-window attention (T=8192, H=16, HK=4, D=128, W=512) on 8 trn2 cores.

Sharding: tensor-parallel on heads. Core c computes query heads {2c, 2c+1},
which share kv head c//2 (G = H//HK = 4, so 2 heads per core never straddle
a kv group). Each core is fully independent -- no collectives.

Per-core program (Bass/Tile, SPMD):
  inputs (host pre-transposed, pre-cast bf16):
    qT  [2*128, T]   Q^T per head (row block j = head j)
    kT  [128, T]     K^T of the shared kv head
    va  [128, nT*129] V chunks [128, 129] with a ones column (chunk t at cols
                     129t..129t+129); the ones column makes the PV matmul also
                     produce the softmax denominator.
  loop over k-chunks t (128 keys each), keys on PSUM partitions:
    S^T[rk, q] = kT_chunk.T @ qT  over the 640-wide valid q-span [128t, 128t+640)
    one ACT exp (scale=D^-0.5 folded in), fp32->bf16, into an SBUF ring
    triangular edge masks (diag block on DVE, border block on GpSimd)
    PV: for j=0..4, lhsT = E block (q-chunk t+j), rhs = va chunk t -> accumulate
        O_aug[q-chunk] = [128, 129] in PSUM (col 128 = denominator)
    retire q-chunk t: copy unnormalized O and denominator to SBUF staging.
  Normalization (out/den) and lse (log den) happen on the host.

The emission is software-pipelined (S matmuls run 2 k-steps ahead of exp) so
the in-order PE queue never stalls on ACT: this keeps PE busy continuously,
which also keeps the PE HAM clock-gate at 2.4 GHz.

PSUM (8 banks): 4 banks = S ring-of-3 [128, 1920]; 4 banks = O accumulators,
5 live slots with staggered lifetimes packed 2 per bank at offsets 0/129 using
(u, u+4) co-tenancy: bank u%4, offset (u//4)%2. start=True (whole-bank
has_written clear) is only ever issued by the offset-0 occupant at its first
touch, which is exactly when the other offset's previous occupant has retired.
"""

import numpy as np
import ml_dtypes

import concourse.bacc as bacc
import concourse.bass as bass
import concourse.mybir as mybir
import concourse.tile as tile
from concourse.bass_utils import run_bass_kernel_spmd

T, H, HK, D, W = 8192, 16, 4, 128, 512
NCORES = 8
SCALE = float(D) ** -0.5
BF16 = ml_dtypes.bfloat16
F32 = mybir.dt.float32
BF = mybir.dt.bfloat16

_NC_CACHE = {}


def _split_at_banks(col0, width):
    """Split [col0, col0+width) PSUM cols at 512 boundaries."""
    pieces = []
    c = col0
    end = col0 + width
    while c < end:
        nxt = min(end, (c // 512 + 1) * 512)
        pieces.append((c, nxt - c))
        c = nxt
    return pieces


def build_program(t_tokens=T):
    nT = t_tokens // 128  # number of 128-wide k/q chunks
    assert t_tokens % 512 == 0 and nT >= 8

    nc = bacc.Bacc("TRN2", target_bir_lowering=False, debug=False,
                   num_devices=NCORES)
    qT_d = nc.dram_tensor("qT", [2 * D, t_tokens], BF, kind="ExternalInput")
    kT_d = nc.dram_tensor("kT", [D, t_tokens], BF, kind="ExternalInput")
    va_d = nc.dram_tensor("va", [D, nT * 129], BF, kind="ExternalInput")
    oaug_d = nc.dram_tensor("oaug", [2, nT // 4, D, 516], F32,
                            kind="ExternalOutput")

    with tile.TileContext(nc) as tc:
        with (
            tc.tile_pool(name="resident", bufs=1) as rpool,
            tc.tile_pool(name="ostg", bufs=2) as ostg_pool,
            tc.tile_pool(name="psum", bufs=1, space="PSUM") as psum_pool,
        ):
            kT = rpool.tile([128, t_tokens], BF, tag="kT")
            va = rpool.tile([128, nT * 129], BF, tag="va")
            qT = [rpool.tile([128, t_tokens], BF, tag=f"qT{j}", name=f"qT{j}")
                  for j in range(2)]
            maskadd = rpool.tile([128, 256], BF, tag="maskadd")
            ident = rpool.tile([128, 128], BF, tag="ident")
            ering = rpool.tile([128, 3840], BF, tag="ering")

            # Separate PSUM tensors so Tile's tensor-granular PSUM-collision
            # serialization never couples the pipelined-ahead S matmuls with
            # the in-flight exp read, nor all PV matmuls with the retire copy.
            srings = [psum_pool.tile([128, 1024], F32, tag=f"sring{i}",
                                     name=f"sring{i}") for i in range(2)]
            oaccs = [psum_pool.tile([128, 512], F32, tag=f"oacc{i}",
                                    name=f"oacc{i}") for i in range(4)]

            # --- constants for PE-side masking ---
            # The triangular edge masks are applied as matmul accumulations
            # onto the S tiles: S_region += maskadd.T @ I adds -1e9 at the
            # invalid positions, so exp underflows them to exactly 0 and no
            # vector-engine masking is needed.
            # maskadd cols [0,128):  A[k, m] = -1e9 where m > k  (diag block)
            # maskadd cols [128,256): B[k, m] = -1e9 where m <= k (border)
            NEG = -1.0e9
            nc.gpsimd.memset(maskadd[:, :], 0.0)
            nc.gpsimd.affine_select(
                out=maskadd[:, 0:128], in_=maskadd[:, 0:128],
                compare_op=mybir.AluOpType.is_ge, fill=NEG,
                base=0, channel_multiplier=1, pattern=[[-1, 128]],
            )
            nc.gpsimd.affine_select(
                out=maskadd[:, 128:256], in_=maskadd[:, 128:256],
                compare_op=mybir.AluOpType.is_ge, fill=NEG,
                base=-1, channel_multiplier=-1, pattern=[[1, 128]],
            )
            nc.gpsimd.memset(ident[:, :], 1.0)
            nc.gpsimd.affine_select(
                out=ident[:, :], in_=ident[:, :],
                compare_op=mybir.AluOpType.is_equal, fill=0.0,
                base=0, channel_multiplier=1, pattern=[[-1, 128]],
            )

            # --- input DMA, sliced and interleaved in first-use order ---
            ks = t_tokens // 4
            vs = (nT * 129) // 4
            qs = t_tokens // 8
            for s in range(4):
                nc.sync.dma_start(kT[:, s * ks:(s + 1) * ks],
                                  kT_d[:, s * ks:(s + 1) * ks])
                nc.sync.dma_start(qT[0][:, s * qs:(s + 1) * qs],
                                  qT_d[0:128, s * qs:(s + 1) * qs])
                nc.sync.dma_start(va[:, s * vs:(s + 1) * vs],
                                  va_d[:, s * vs:(s + 1) * vs])
            for s in range(4, 8):
                nc.sync.dma_start(qT[0][:, s * qs:(s + 1) * qs],
                                  qT_d[0:128, s * qs:(s + 1) * qs])
            for s in range(8):
                nc.sync.dma_start(qT[1][:, s * qs:(s + 1) * qs],
                                  qT_d[128:256, s * qs:(s + 1) * qs])

            def s_matmuls(j, t):
                """S^T matmuls for k-chunk t into ring slice t%2.

                The 1024-col ring stride keeps the two slices in disjoint
                PSUM bank pairs, so the exp read of slice t%2 never shares a
                bank with the pipelined-ahead S write of slice (t+1)%2.
                Main and tail matmuls stay open (stop=False) so the
                triangular -1e9 mask-add matmuls can accumulate into them
                and close the groups.
                """
                sring = srings[t % 2]
                k0 = 128 * t
                wm = min(512, t_tokens - k0)
                nc.tensor.matmul(sring[:, 0:wm],
                                 kT[:, k0:k0 + 128],
                                 qT[j][:, k0:k0 + wm],
                                 start=True, stop=False)
                nc.tensor.matmul(sring[:, 0:128],
                                 maskadd[:, 0:128], ident[:, :],
                                 start=False, stop=True)
                wt = min(128, max(0, t_tokens - k0 - 512))
                if wt > 0:
                    nc.tensor.matmul(sring[:, 512:512 + wt],
                                     kT[:, k0:k0 + 128],
                                     qT[j][:, k0 + 512:k0 + 512 + wt],
                                     start=True, stop=False)
                    nc.tensor.matmul(sring[:, 512:512 + wt],
                                     maskadd[:, 128:256], ident[:, 0:wt],
                                     start=False, stop=True)
                return wm + wt

            def width_of(t):
                k0 = 128 * t
                return min(512, t_tokens - k0) + min(128, max(0, t_tokens - k0 - 512))

            def emit_exp(t):
                ebase = 640 * (t % 6)
                w = width_of(t)
                nc.scalar.activation(ering[:, ebase:ebase + w],
                                     srings[t % 2][:, 0:w],
                                     mybir.ActivationFunctionType.Exp,
                                     scale=SCALE)

            def mask_and_pv(j, t):
                base = 640 * (t % 6)
                jmax = min(4, nT - 1 - t)
                # j0 before j4: same-bank co-tenancy requires the retiring
                # slot's last accumulation before the whole-bank clearing
                # start=True of slot u+4. j3 last: it shares a bank with the
                # previous step's retire copy, so issuing it last hides that
                # serialization behind the other PV matmuls.
                order = [jj for jj in (0, 1, 2, 4, 3) if jj <= jmax]
                for jj in order:
                    u = t + jj
                    off = 129 * ((u // 4) % 2)
                    first = (jj == 4) or (t == 0)
                    start = first and ((u // 4) % 2 == 0)
                    # Co-tenant accumulators share banks; the sim's
                    # bank-granular group checker can't express this, but its
                    # per-byte pending-zero value model (== HW has_written)
                    # verifies the numerics.
                    nc.tensor.matmul(
                        oaccs[u % 4][:, off:off + 129],
                        ering[:, base + 128 * jj:base + 128 * jj + 128],
                        va[:, 129 * t:129 * t + 129],
                        start=start, stop=(jj == 0),
                        skip_group_check=True)

            ostage = [None]

            def retire(j, t):
                u = t
                off = 129 * ((u // 4) % 2)
                if t % 4 == 0:
                    ostage[0] = ostg_pool.tile([128, 516], F32, tag="ostage",
                                               name="ostage")
                nc.vector.tensor_copy(
                    ostage[0][:, 129 * (t % 4):129 * (t % 4) + 129],
                    oaccs[u % 4][:, off:off + 129])
                if t % 4 == 3:
                    nc.sync.dma_start(oaug_d[j, t // 4], ostage[0][:, :])

            # software-pipelined emission: S runs 1 k-step ahead of exp
            for j in range(2):
                s_matmuls(j, 0)
                for t in range(nT):
                    emit_exp(t)
                    if t + 1 < nT:
                        s_matmuls(j, t + 1)
                    mask_and_pv(j, t)
                    retire(j, t)

    nc.compile()
    return nc


def _get_nc(t_tokens=T):
    if t_tokens not in _NC_CACHE:
        _NC_CACHE[t_tokens] = build_program(t_tokens)
    return _NC_CACHE[t_tokens]


def make_in_maps(query, key, value, t_tokens=T):
    q = np.asarray(query).astype(BF16).reshape(t_tokens, H, D)
    k = np.asarray(key).astype(BF16).reshape(t_tokens, HK, D)
    v = np.asarray(value).astype(BF16).reshape(t_tokens, HK, D)
    nT = t_tokens // 128
    in_maps = []
    for c in range(NCORES):
        h0, hk = 2 * c, c // 2
        qT = np.ascontiguousarray(
            q[:, h0:h0 + 2, :].transpose(1, 2, 0)).reshape(2 * D, t_tokens)
        kT = np.ascontiguousarray(k[:, hk, :].T)
        vv = v[:, hk, :].reshape(nT, 128, D).transpose(1, 0, 2)
        va = np.empty((128, nT, D + 1), dtype=BF16)
        va[:, :, :D] = vv
        va[:, :, D] = 1.0
        in_maps.append({"qT": qT, "kT": kT, "va": va.reshape(128, nT * 129)})
    return in_maps


def assemble(results, t_tokens=T):
    nT = t_tokens // 128
    out = np.empty((t_tokens, H * D), np.float32)
    lse = np.empty((H, t_tokens), np.float32)
    for c in range(NCORES):
        oaug = results[c]["oaug"]  # [2, nT//4, 128, 516]
        for j in range(2):
            a = oaug[j].reshape(nT // 4, 128, 4, 129)
            a = a.transpose(0, 2, 1, 3).reshape(t_tokens, 129)
            d_q = a[:, 128]
            cols = slice(256 * c + 128 * j, 256 * c + 128 * j + 128)
            out[:, cols] = a[:, :128] / d_q[:, None]
            lse[2 * c + j] = np.log(d_q)
    return out, lse


def kernel(query, key, value):
    nc = _get_nc(T)
    in_maps = make_in_maps(query, key, value, T)
    res = run_bass_kernel_spmd(nc, in_maps, list(range(NCORES)))
    return assemble(res.results, T)
